# revision 17
# baseline (speedup 1.0000x reference)
"""Trainium2 Bass kernel for CifNet conv-QKV self-attention.

Sharding: 8 cores = 4 (batch) x 2 (head-groups of 4 heads).
Each core computes, for its batch sample b and head-group g:
  - q/k/v = conv3x3(x, w{q,k,v}[g*256:(g+1)*256])   (256 out-channels = 4 heads)
  - per-head attention over hw=2304 positions (softmax without max-subtraction,
    denominator fused into the AV matmul via an appended ones-column on V^T)
  - partial o-conv: conv3x3(attn_out, wo[:, g*256:(g+1)*256])  -> [256, 2304] fp32
Host sums the two head-group partials per batch sample.

Convs are expressed as 9 shifted matmuls (one per tap) accumulating in PSUM,
with the input pre-padded to [C, 50, 50] on the host. All matmuls run in bf16
with fp32 PSUM accumulation (measured end-to-end rel-l2 error ~5.5e-3).
"""

from contextlib import ExitStack

import numpy as np
import ml_dtypes

# problem shape (hardcoded per contract)
B, C, H, W = 4, 256, 48, 48
HW = H * W              # 2304
NCORES = 8
RT = 8                  # output rows per spatial tile
NT = RT * W             # 384 columns per matmul
NROW = H // RT          # 6 spatial tiles
NKJ = HW // 128         # 18 key tiles
KJG = 3                 # kj tiles per exp group
NGRP = NKJ // KJG       # 6 groups

_cached = None


def _build():
    """Build and compile the per-core SPMD Bass program (cached)."""
    global _cached
    if _cached is not None:
        return _cached

    import concourse.bass as bass  # noqa: F401
    import concourse.tile as tile
    from concourse import bacc, mybir

    BF = mybir.dt.bfloat16
    F32 = mybir.dt.float32
    EXP = mybir.ActivationFunctionType.Exp

    nc = bacc.Bacc("TRN2", target_bir_lowering=False, debug=False)
    x_d = nc.dram_tensor("xpad", [2, 128, 50, 50], BF, kind="ExternalInput").ap()
    wqkv_d = nc.dram_tensor("wqkv", [3, 9, 2, 128, 256], BF, kind="ExternalInput").ap()
    wo_d = nc.dram_tensor("wo", [9, 2, 128, 256], BF, kind="ExternalInput").ap()
    out_d = nc.dram_tensor("out", [2, 128, HW], F32, kind="ExternalOutput").ap()

    with tile.TileContext(nc) as tc, ExitStack() as ctx:
        konst = ctx.enter_context(tc.tile_pool(name="konst", bufs=1))
        x_sb = konst.tile([128, 2, 50, 50], BF, name="x_sb")
        wq_sb = konst.tile([128, 9, 2, 256], BF, name="wq_sb")
        wk_sb = konst.tile([128, 9, 2, 256], BF, name="wk_sb")
        wv_sb = konst.tile([128, 9, 2, 256], BF, name="wv_sb")
        wo_sb = konst.tile([128, 9, 2, 256], BF, name="wo_sb")
        q_sb = [konst.tile([128, HW], BF, name=f"q_sb{m}") for m in range(2)]
        k_sb = [konst.tile([128, HW], BF, name=f"k_sb{m}") for m in range(2)]
        v_sb = [konst.tile([128, HW], BF, name=f"v_sb{m}") for m in range(2)]
        # V^T per head: [kj within tile, kj tile, 128]; col 64 holds ones so
        # the AV matmul also produces the softmax denominator in psum row 64.
        # Padded to 128 cols so each xbar-transpose dst is 128B-aligned.
        vt_sb = [konst.tile([128, NKJ, 128], BF, name=f"vt_sb{h}") for h in range(4)]
        opad = [konst.tile([128, 50, 50], BF, name=f"opad{g}") for g in range(2)]

        # input DMAs, in consumption order: the first conv matmul needs only
        # x kg0 + wv tap0, and x kg1 isn't consumed until 54 matmuls in
        nc.sync.dma_start(x_sb[:, 0], x_d[0])
        for t in range(9):
            nc.sync.dma_start(wv_sb[:, t], wqkv_d[2, t].rearrange("g p o -> p g o"))
        nc.sync.dma_start(x_sb[:, 1], x_d[1])
        for a, w_sb in ((0, wq_sb), (1, wk_sb)):
            for t in range(9):
                nc.sync.dma_start(w_sb[:, t], wqkv_d[a, t].rearrange("g p o -> p g o"))
        for t in range(9):
            nc.sync.dma_start(wo_sb[:, t], wo_d[t].rearrange("g p o -> p g o"))

        for h in range(4):
            nc.gpsimd.memset(vt_sb[h][:], 1.0)
        for g in range(2):
            nc.gpsimd.memset(opad[g][:], 0.0)

        # warm the ACT exp table during the DMA phase (one-time ~2.7us load)
        wrm = konst.tile([1, 8], F32, name="wrm")
        nc.gpsimd.memset(wrm[:], 0.0)
        nc.scalar.activation(wrm[:], wrm[:], EXP, scale=0.125)

        # ---------------- phase A: m0 convs + v-m0 transposes ----------------
        def conv_block(m, w_sb, dst, cpool, x_src):
            """One full conv output tile-row group: 18 accumulating MMs x 6 rowtiles."""
            ps = [cpool.tile([128, NT], F32, tag="cps", name="cps") for _ in range(NROW)]
            first = True
            for kg in range(2):
                for t in range(9):
                    ky, kx = t // 3, t % 3
                    lhsT = w_sb[:, t, kg, m * 128:(m + 1) * 128]
                    last = (kg == 1 and t == 8)
                    for r in range(NROW):
                        rhs = x_src[:, kg, r * RT + ky: r * RT + ky + RT, kx: kx + W]
                        nc.tensor.matmul(ps[r][:], lhsT, rhs, start=first, stop=last)
                    first = False
            for r in range(NROW):
                nc.vector.tensor_copy(dst[:, r * NT:(r + 1) * NT], ps[r][:])

        def transpose_v(m):
            """V^T via DMA xbar transpose (off the PE): 36 tile transposes."""
            for hh in range(2):
                h = 2 * m + hh
                for kt in range(NKJ):
                    nc.sync.dma_start(
                        vt_sb[h][:, kt, 0:64],
                        v_sb[m][64 * hh:64 * hh + 64, kt * 128:(kt + 1) * 128],
                        transpose=True,
                    )

        with tc.tile_pool(name="cpsum", bufs=6, space="PSUM") as cpsum:
            conv_block(0, wv_sb, v_sb[0], cpsum, x_sb)
            transpose_v(0)
            conv_block(0, wq_sb, q_sb[0], cpsum, x_sb)
            conv_block(0, wk_sb, k_sb[0], cpsum, x_sb)

        # ---------------- phases B/C/D: attention interleaved with m1 convs
        # and the o-conv, so the PE always has independent work and never
        # blips waiting on the ACT exp (which would throttle its clock).
        osum = [konst.tile([128, HW], F32, name=f"osum{mo}") for mo in range(2)]

        with tc.tile_pool(name="spsum", bufs=2, space="PSUM") as spsum, \
             tc.tile_pool(name="apsum", bufs=2, space="PSUM") as apsum, \
             tc.tile_pool(name="fpsum", bufs=2, space="PSUM") as fpsum, \
             tc.tile_pool(name="esb", bufs=4) as esb, \
             tc.tile_pool(name="osb", bufs=3) as osb, \
             tc.tile_pool(name="nsb", bufs=2) as nsb:

            def conv_row_unit(m, w_sb, dst, r):
                """One rowtile of a conv: 18 accumulating MMs into 1 psum bank."""
                ps = fpsum.tile([128, NT], F32, tag="fps", name="fps")
                first = True
                for kg in range(2):
                    for t in range(9):
                        ky, kx = t // 3, t % 3
                        lhsT = w_sb[:, t, kg, m * 128:(m + 1) * 128]
                        rhs = x_sb[:, kg, r * RT + ky: r * RT + ky + RT, kx: kx + W]
                        nc.tensor.matmul(ps[:], lhsT, rhs, start=first,
                                         stop=(kg == 1 and t == 8))
                        first = False
                nc.vector.tensor_copy(dst[:, r * NT:(r + 1) * NT], ps[:])

            def oconv_row_unit(mo, r, kg):
                """One rowtile of the o-conv for one input kgroup (9 taps)."""
                ps = fpsum.tile([128, NT], F32, tag="fps", name="fps")
                for t in range(9):
                    ky, kx = t // 3, t % 3
                    lhsT = wo_sb[:, t, kg, mo * 128:(mo + 1) * 128]
                    rhs = opad[kg][:, r * RT + ky: r * RT + ky + RT, kx: kx + W]
                    nc.tensor.matmul(ps[:], lhsT, rhs, start=(t == 0), stop=(t == 8))
                if kg == 0:
                    nc.vector.tensor_copy(osum[mo][:, r * NT:(r + 1) * NT], ps[:])
                else:
                    ot = osb.tile([128, NT], F32, tag="osb", name="osb")
                    nc.vector.tensor_tensor(
                        ot[:], ps[:], osum[mo][:, r * NT:(r + 1) * NT],
                        mybir.AluOpType.add,
                    )
                    nc.sync.dma_start(out_d[mo, :, r * NT:(r + 1) * NT], ot[:])

            def att_unit(m, qi, grp2):
                """Both heads / 2 kj tiles: 4 row-packed score MMs (adjacent
                (0,0)/(64,0) pairs overlap in the array), 2 exps, 4 AV MMs."""
                qsl = slice(qi * NT, (qi + 1) * NT)
                sp = [spsum.tile([128, 2, 512], F32, tag="sps", name="sps")
                      for _ in range(2)]
                for j in range(2):
                    kjt = grp2 * 2 + j
                    for hh in range(2):
                        nc.tensor.matmul(
                            sp[hh][:, j, 0:NT],
                            k_sb[m][64 * hh:64 * hh + 64, kjt * 128:(kjt + 1) * 128],
                            q_sb[m][64 * hh:64 * hh + 64, qsl],
                            start=True, stop=True,
                            tile_position=(64 * hh, 0),
                        )
                ets = []
                for hh in range(2):
                    et = esb.tile([128, 2, NT], BF, tag="et", name="et")
                    nc.scalar.activation(et[:], sp[hh][:, :, 0:NT], EXP, scale=0.125)
                    ets.append(et)
                for hh in range(2):
                    h = 2 * m + hh
                    for j in range(2):
                        kjt = grp2 * 2 + j
                        nc.tensor.matmul(
                            av_cur[hh][0:65, :], vt_sb[h][:, kjt, 0:65],
                            ets[hh][:, j, :],
                            start=(kjt == 0), stop=(kjt == NKJ - 1),
                        )

            def normalize(m, qi, hh):
                avf = nsb.tile([128, NT], F32, tag="avf", name="avf")
                nc.vector.tensor_copy(avf[0:65, :], av_cur[hh][0:65, :])
                dn = nsb.tile([1, NT], F32, tag="dn", name="dn")
                nc.sync.dma_start(dn[:], avf[64:65, :])
                rc = nsb.tile([1, NT], F32, tag="rc", name="rc")
                nc.vector.reciprocal_approx_fast(rc[:], dn[:])
                rb = nsb.tile([64, NT], F32, tag="rb", name="rb")
                nc.gpsimd.partition_broadcast(rb[:], rc[:])
                tmp = nsb.tile([64, NT], BF, tag="tmp", name="tmp")
                nc.vector.tensor_mul(tmp[:], avf[0:64, :], rb[:])
                dst = opad[m][64 * hh:64 * hh + 64, qi * RT + 1: qi * RT + RT + 1, 1:49]
                nc.sync.dma_start(dst, tmp[:].rearrange("p (r c) -> p r c", c=W))

            # filler list: PE-only work dripped into the attention stream, in
            # dependency order (v conv first, then its transposes, then q/k)
            fillers_b = (
                [lambda r=r: conv_row_unit(1, wv_sb, v_sb[1], r) for r in range(NROW)]
                + [lambda: transpose_v(1)]
                + [lambda r=r: conv_row_unit(1, wq_sb, q_sb[1], r) for r in range(NROW)]
                + [lambda r=r: conv_row_unit(1, wk_sb, k_sb[1], r) for r in range(NROW)]
            )

            def run_attention(m, fillers):
                """Emit all attention units for head-pair m, interspersing
                fillers; drip finishes ~2 units early so the next phase's
                dependencies are ready at the boundary."""
                fi = 0
                n_units = NROW * 9 - 2
                ui = 0
                for qi in range(NROW):
                    av_cur[0] = apsum.tile([128, NT], F32, tag="avps", name="avps")
                    av_cur[1] = apsum.tile([128, NT], F32, tag="avps", name="avps")
                    for grp2 in range(9):
                        att_unit(m, qi, grp2)
                        ui += 1
                        # drip PE-only work at a steady rate
                        while fi < len(fillers) and ui * len(fillers) >= (fi + 1) * n_units:
                            fillers[fi]()
                            fi += 1
                    for hh in range(2):
                        normalize(m, qi, hh)
                while fi < len(fillers):
                    fillers[fi]()
                    fi += 1

            av_cur = [None, None]
            run_attention(0, fillers_b)

            # phase C: attention m1, interleaved with the o-conv kg0 partial
            # pass (rows 0..3 only) and the early rows of the kg1 pass (kg1
            # row r only needs opad[1] rows <= r*8+10, i.e. normalize(1,
            # qi<=r+1) done; the filler drip rate places it well after that).
            fillers_c = (
                [lambda mo=mo, r=r: oconv_row_unit(mo, r, 0)
                 for mo in range(2) for r in range(NROW - 3)]
                + [lambda mo=mo, r=r: oconv_row_unit(mo, r, 1)
                   for r in range(NROW - 3) for mo in range(2)]
            )
            run_attention(1, fillers_c)

            # phase D: kg0 rows 3-5 first — they depend only on phase-B data,
            # so they keep the PE busy while the last normalizes (qi 4/5 of
            # m1) drain; then the kg1 rows that need those normalizes.
            for r in range(NROW - 3, NROW):
                for mo in range(2):
                    oconv_row_unit(mo, r, 0)
            for r in range(NROW - 3, NROW):
                for mo in range(2):
                    oconv_row_unit(mo, r, 1)

    nc.compile()
    _cached = nc
    return nc


def make_in_maps(hidden_states, wq, wk, wv, wo):
    """Shard + pre-transform full inputs into 8 per-core input dicts."""
    bf = ml_dtypes.bfloat16
    hidden_states = np.asarray(hidden_states, np.float32)
    in_maps = []
    for core in range(NCORES):
        b, g = core // 2, core % 2
        xp = np.zeros((C, 50, 50), np.float32)
        xp[:, 1:49, 1:49] = hidden_states[b]
        xpad = np.ascontiguousarray(xp.reshape(2, 128, 50, 50)).astype(bf)
        wstk = np.stack(
            [
                np.asarray(w, np.float32)[g * 256:(g + 1) * 256]
                .transpose(2, 3, 1, 0)
                .reshape(9, 2, 128, 256)
                for w in (wq, wk, wv)
            ]
        ).astype(bf)
        wog = (
            np.asarray(wo, np.float32)[:, g * 256:(g + 1) * 256]
            .transpose(2, 3, 1, 0)
            .reshape(9, 2, 128, 256)
            .astype(bf)
        )
        in_maps.append({"xpad": xpad, "wqkv": wstk, "wo": wog})
    return in_maps


def combine_outputs(per_core_outs):
    """Sum the two head-group partials per batch sample."""
    out = np.empty((B, C, H, W), np.float32)
    for b in range(B):
        acc = per_core_outs[2 * b].reshape(C, HW).astype(np.float32) + \
              per_core_outs[2 * b + 1].reshape(C, HW).astype(np.float32)
        out[b] = acc.reshape(C, H, W)
    return out


def kernel(hidden_states, wq, wk, wv, wo):
    from concourse.bass_utils import run_bass_kernel_spmd

    nc = _build()
    in_maps = make_in_maps(hidden_states, wq, wk, wv, wo)
    res = run_bass_kernel_spmd(nc, in_maps, core_ids=list(range(NCORES)))
    return combine_outputs([r["out"] for r in res.results])



# revision 20
# speedup vs baseline: 1.0147x; 1.0147x over previous
"""Trainium2 Bass kernel for CifNet conv-QKV self-attention.

Sharding: 8 cores = 4 (batch) x 2 (head-groups of 4 heads).
Each core computes, for its batch sample b and head-group g:
  - q/k/v = conv3x3(x, w{q,k,v}[g*256:(g+1)*256])   (256 out-channels = 4 heads)
  - per-head attention over hw=2304 positions (softmax without max-subtraction,
    denominator fused into the AV matmul via an appended ones-column on V^T)
  - partial o-conv: conv3x3(attn_out, wo[:, g*256:(g+1)*256])  -> [256, 2304] fp32
Host sums the two head-group partials per batch sample.

Convs are expressed as 9 shifted matmuls (one per tap) accumulating in PSUM,
with the input pre-padded to [C, 50, 50] on the host. All matmuls run in bf16
with fp32 PSUM accumulation (measured end-to-end rel-l2 error ~5.5e-3).
"""

from contextlib import ExitStack

import numpy as np
import ml_dtypes

# problem shape (hardcoded per contract)
B, C, H, W = 4, 256, 48, 48
HW = H * W              # 2304
NCORES = 8
RT = 8                  # output rows per spatial tile
NT = RT * W             # 384 columns per matmul
NROW = H // RT          # 6 spatial tiles
NKJ = HW // 128         # 18 key tiles
KJG = 3                 # kj tiles per exp group
NGRP = NKJ // KJG       # 6 groups

_cached = None


def _build():
    """Build and compile the per-core SPMD Bass program (cached)."""
    global _cached
    if _cached is not None:
        return _cached

    import concourse.bass as bass  # noqa: F401
    import concourse.tile as tile
    from concourse import bacc, mybir

    BF = mybir.dt.bfloat16
    F32 = mybir.dt.float32
    EXP = mybir.ActivationFunctionType.Exp

    nc = bacc.Bacc("TRN2", target_bir_lowering=False, debug=False)
    x_d = nc.dram_tensor("xpad", [2, 128, 50, 50], BF, kind="ExternalInput").ap()
    wqkv_d = nc.dram_tensor("wqkv", [3, 9, 2, 128, 256], BF, kind="ExternalInput").ap()
    wo_d = nc.dram_tensor("wo", [9, 2, 128, 256], BF, kind="ExternalInput").ap()
    out_d = nc.dram_tensor("out", [2, 128, HW], F32, kind="ExternalOutput").ap()

    with tile.TileContext(nc) as tc, ExitStack() as ctx:
        konst = ctx.enter_context(tc.tile_pool(name="konst", bufs=1))
        x_sb = konst.tile([128, 2, 50, 50], BF, name="x_sb")
        wq_sb = konst.tile([128, 9, 2, 256], BF, name="wq_sb")
        wk_sb = konst.tile([128, 9, 2, 256], BF, name="wk_sb")
        wv_sb = konst.tile([128, 9, 2, 256], BF, name="wv_sb")
        wo_sb = konst.tile([128, 9, 2, 256], BF, name="wo_sb")
        q_sb = [konst.tile([128, HW], BF, name=f"q_sb{m}") for m in range(2)]
        k_sb = [konst.tile([128, HW], BF, name=f"k_sb{m}") for m in range(2)]
        v_sb = [konst.tile([128, HW], BF, name=f"v_sb{m}") for m in range(2)]
        # V^T per head: [kj within tile, kj tile, 128]; col 64 holds ones so
        # the AV matmul also produces the softmax denominator in psum row 64.
        # Padded to 128 cols so each xbar-transpose dst is 128B-aligned.
        vt_sb = [konst.tile([128, NKJ, 128], BF, name=f"vt_sb{h}") for h in range(4)]
        opad = [konst.tile([128, 50, 50], BF, name=f"opad{g}") for g in range(2)]

        # input DMAs, in consumption order: the first conv matmul needs only
        # x kg0 + wv tap0, and x kg1 isn't consumed until 54 matmuls in
        nc.sync.dma_start(x_sb[:, 0], x_d[0])
        for t in range(9):
            nc.sync.dma_start(wv_sb[:, t], wqkv_d[2, t].rearrange("g p o -> p g o"))
        nc.sync.dma_start(x_sb[:, 1], x_d[1])
        for a, w_sb in ((0, wq_sb), (1, wk_sb)):
            for t in range(9):
                nc.sync.dma_start(w_sb[:, t], wqkv_d[a, t].rearrange("g p o -> p g o"))
        for t in range(9):
            nc.sync.dma_start(wo_sb[:, t], wo_d[t].rearrange("g p o -> p g o"))

        for h in range(4):
            nc.gpsimd.memset(vt_sb[h][:], 1.0)
        for g in range(2):
            nc.gpsimd.memset(opad[g][:], 0.0)

        # warm the ACT exp table during the DMA phase (one-time ~2.7us load)
        wrm = konst.tile([1, 8], F32, name="wrm")
        nc.gpsimd.memset(wrm[:], 0.0)
        nc.scalar.activation(wrm[:], wrm[:], EXP, scale=0.125)

        # ---------------- phase A: m0 convs + v-m0 transposes ----------------
        def conv_block(m, w_sb, dst, cpool, x_src):
            """One full conv output tile-row group: 18 accumulating MMs x 6 rowtiles."""
            ps = [cpool.tile([128, NT], F32, tag="cps", name="cps") for _ in range(NROW)]
            first = True
            for kg in range(2):
                for t in range(9):
                    ky, kx = t // 3, t % 3
                    lhsT = w_sb[:, t, kg, m * 128:(m + 1) * 128]
                    last = (kg == 1 and t == 8)
                    for r in range(NROW):
                        rhs = x_src[:, kg, r * RT + ky: r * RT + ky + RT, kx: kx + W]
                        nc.tensor.matmul(ps[r][:], lhsT, rhs, start=first, stop=last)
                    first = False
            for r in range(NROW):
                nc.vector.tensor_copy(dst[:, r * NT:(r + 1) * NT], ps[r][:])

        def transpose_v(m):
            """V^T via DMA xbar transpose (off the PE): 36 tile transposes."""
            for hh in range(2):
                h = 2 * m + hh
                for kt in range(NKJ):
                    nc.sync.dma_start(
                        vt_sb[h][:, kt, 0:64],
                        v_sb[m][64 * hh:64 * hh + 64, kt * 128:(kt + 1) * 128],
                        transpose=True,
                    )

        with tc.tile_pool(name="cpsum", bufs=6, space="PSUM") as cpsum:
            conv_block(0, wv_sb, v_sb[0], cpsum, x_sb)
            transpose_v(0)
            conv_block(0, wq_sb, q_sb[0], cpsum, x_sb)
            conv_block(0, wk_sb, k_sb[0], cpsum, x_sb)

        # ---------------- phases B/C/D: attention interleaved with m1 convs
        # and the o-conv, so the PE always has independent work and never
        # blips waiting on the ACT exp (which would throttle its clock).
        osum = [konst.tile([128, HW], F32, name=f"osum{mo}") for mo in range(2)]

        with tc.tile_pool(name="spsum", bufs=2, space="PSUM") as spsum, \
             tc.tile_pool(name="apsum", bufs=2, space="PSUM") as apsum, \
             tc.tile_pool(name="fpsum", bufs=2, space="PSUM") as fpsum, \
             tc.tile_pool(name="esb", bufs=4) as esb, \
             tc.tile_pool(name="osb", bufs=3) as osb, \
             tc.tile_pool(name="nsb", bufs=2) as nsb:

            def conv_row_unit(m, w_sb, dst, r):
                """One rowtile of a conv: 18 accumulating MMs into 1 psum bank."""
                ps = fpsum.tile([128, NT], F32, tag="fps", name="fps")
                first = True
                for kg in range(2):
                    for t in range(9):
                        ky, kx = t // 3, t % 3
                        lhsT = w_sb[:, t, kg, m * 128:(m + 1) * 128]
                        rhs = x_sb[:, kg, r * RT + ky: r * RT + ky + RT, kx: kx + W]
                        nc.tensor.matmul(ps[:], lhsT, rhs, start=first,
                                         stop=(kg == 1 and t == 8))
                        first = False
                nc.vector.tensor_copy(dst[:, r * NT:(r + 1) * NT], ps[:])

            def oconv_row_unit(mo, r, kg):
                """One rowtile of the o-conv for one input kgroup (9 taps)."""
                ps = fpsum.tile([128, NT], F32, tag="fps", name="fps")
                for t in range(9):
                    ky, kx = t // 3, t % 3
                    lhsT = wo_sb[:, t, kg, mo * 128:(mo + 1) * 128]
                    rhs = opad[kg][:, r * RT + ky: r * RT + ky + RT, kx: kx + W]
                    nc.tensor.matmul(ps[:], lhsT, rhs, start=(t == 0), stop=(t == 8))
                if kg == 0:
                    nc.vector.tensor_copy(osum[mo][:, r * NT:(r + 1) * NT], ps[:])
                else:
                    ot = osb.tile([128, NT], F32, tag="osb", name="osb")
                    nc.vector.tensor_tensor(
                        ot[:], ps[:], osum[mo][:, r * NT:(r + 1) * NT],
                        mybir.AluOpType.add,
                    )
                    nc.sync.dma_start(out_d[mo, :, r * NT:(r + 1) * NT], ot[:])

            def att_unit(m, qi, grp2):
                """Both heads / 2 kj tiles: 4 row-packed score MMs (adjacent
                (0,0)/(64,0) pairs overlap in the array), 2 exps, 4 AV MMs."""
                qsl = slice(qi * NT, (qi + 1) * NT)
                sp = [spsum.tile([128, 2, 512], F32, tag="sps", name="sps")
                      for _ in range(2)]
                for j in range(2):
                    kjt = grp2 * 2 + j
                    for hh in range(2):
                        nc.tensor.matmul(
                            sp[hh][:, j, 0:NT],
                            k_sb[m][64 * hh:64 * hh + 64, kjt * 128:(kjt + 1) * 128],
                            q_sb[m][64 * hh:64 * hh + 64, qsl],
                            start=True, stop=True,
                            tile_position=(64 * hh, 0),
                        )
                ets = []
                for hh in range(2):
                    et = esb.tile([128, 2, NT], BF, tag="et", name="et")
                    nc.scalar.activation(et[:], sp[hh][:, :, 0:NT], EXP, scale=0.125)
                    ets.append(et)
                for hh in range(2):
                    h = 2 * m + hh
                    for j in range(2):
                        kjt = grp2 * 2 + j
                        nc.tensor.matmul(
                            av_cur[hh][0:65, :], vt_sb[h][:, kjt, 0:65],
                            ets[hh][:, j, :],
                            start=(kjt == 0), stop=(kjt == NKJ - 1),
                        )

            def normalize(m, qi, hh):
                avf = nsb.tile([128, NT], F32, tag="avf", name="avf")
                nc.vector.tensor_copy(avf[0:65, :], av_cur[hh][0:65, :])
                dn = nsb.tile([1, NT], F32, tag="dn", name="dn")
                nc.sync.dma_start(dn[:], avf[64:65, :])
                rc = nsb.tile([1, NT], F32, tag="rc", name="rc")
                nc.vector.reciprocal_approx_fast(rc[:], dn[:])
                rb = nsb.tile([64, NT], F32, tag="rb", name="rb")
                nc.gpsimd.partition_broadcast(rb[:], rc[:])
                tmp = nsb.tile([64, NT], BF, tag="tmp", name="tmp")
                nc.vector.tensor_mul(tmp[:], avf[0:64, :], rb[:])
                dst = opad[m][64 * hh:64 * hh + 64, qi * RT + 1: qi * RT + RT + 1, 1:49]
                nc.sync.dma_start(dst, tmp[:].rearrange("p (r c) -> p r c", c=W))

            # filler list: PE-only work dripped into the attention stream, in
            # dependency order (v conv first, then its transposes, then q/k)
            fillers_b = (
                [lambda r=r: conv_row_unit(1, wv_sb, v_sb[1], r) for r in range(NROW)]
                + [lambda: transpose_v(1)]
                + [lambda r=r: conv_row_unit(1, wq_sb, q_sb[1], r) for r in range(NROW)]
                + [lambda r=r: conv_row_unit(1, wk_sb, k_sb[1], r)
                   for r in range(NROW - 2)]
            )

            def run_attention(m, fillers):
                """Emit all attention units for head-pair m, interspersing fillers."""
                fi = 0
                n_units = NROW * 9
                ui = 0
                for qi in range(NROW):
                    av_cur[0] = apsum.tile([128, NT], F32, tag="avps", name="avps")
                    av_cur[1] = apsum.tile([128, NT], F32, tag="avps", name="avps")
                    for grp2 in range(9):
                        att_unit(m, qi, grp2)
                        ui += 1
                        # drip PE-only work at a steady rate
                        while fi < len(fillers) and ui * len(fillers) >= (fi + 1) * n_units:
                            fillers[fi]()
                            fi += 1
                    for hh in range(2):
                        normalize(m, qi, hh)
                while fi < len(fillers):
                    fillers[fi]()
                    fi += 1

            av_cur = [None, None]
            run_attention(0, fillers_b)

            # phase C: attention m1, interleaved with the o-conv kg0 partial
            # pass (rows 0..3 only) and the early rows of the kg1 pass (kg1
            # row r only needs opad[1] rows <= r*8+10, i.e. normalize(1,
            # qi<=r+1) done; the filler drip rate places it well after that).
            # the last two k-conv m1 rowtiles lead phase C's fillers: phase C
            # is ACT(exp)-bound at the margin, so it takes the extra PE work
            # from the PE-bound phase B; units 1-8 of phase C only touch k
            # columns from rows 0-4, and these fillers complete by unit ~7.
            fillers_c = (
                [lambda r=r: conv_row_unit(1, wk_sb, k_sb[1], r)
                 for r in range(NROW - 2, NROW)]
                + [lambda mo=mo, r=r: oconv_row_unit(mo, r, 0)
                   for mo in range(2) for r in range(NROW - 2)]
                + [lambda mo=mo, r=r: oconv_row_unit(mo, r, 1)
                   for r in range(NROW - 2) for mo in range(2)]
            )
            run_attention(1, fillers_c)

            # phase D: kg0 rows 4-5 first — they depend only on phase-B data,
            # so they keep the PE busy while the last normalizes (qi 4/5 of
            # m1) drain; then the kg1 rows that need those normalizes.
            for r in range(NROW - 2, NROW):
                for mo in range(2):
                    oconv_row_unit(mo, r, 0)
            for r in range(NROW - 2, NROW):
                for mo in range(2):
                    oconv_row_unit(mo, r, 1)

    nc.compile()
    _cached = nc
    return nc


def make_in_maps(hidden_states, wq, wk, wv, wo):
    """Shard + pre-transform full inputs into 8 per-core input dicts."""
    bf = ml_dtypes.bfloat16
    hidden_states = np.asarray(hidden_states, np.float32)
    in_maps = []
    for core in range(NCORES):
        b, g = core // 2, core % 2
        xp = np.zeros((C, 50, 50), np.float32)
        xp[:, 1:49, 1:49] = hidden_states[b]
        xpad = np.ascontiguousarray(xp.reshape(2, 128, 50, 50)).astype(bf)
        wstk = np.stack(
            [
                np.asarray(w, np.float32)[g * 256:(g + 1) * 256]
                .transpose(2, 3, 1, 0)
                .reshape(9, 2, 128, 256)
                for w in (wq, wk, wv)
            ]
        ).astype(bf)
        wog = (
            np.asarray(wo, np.float32)[:, g * 256:(g + 1) * 256]
            .transpose(2, 3, 1, 0)
            .reshape(9, 2, 128, 256)
            .astype(bf)
        )
        in_maps.append({"xpad": xpad, "wqkv": wstk, "wo": wog})
    return in_maps


def combine_outputs(per_core_outs):
    """Sum the two head-group partials per batch sample."""
    out = np.empty((B, C, H, W), np.float32)
    for b in range(B):
        acc = per_core_outs[2 * b].reshape(C, HW).astype(np.float32) + \
              per_core_outs[2 * b + 1].reshape(C, HW).astype(np.float32)
        out[b] = acc.reshape(C, H, W)
    return out


def kernel(hidden_states, wq, wk, wv, wo):
    from concourse.bass_utils import run_bass_kernel_spmd

    nc = _build()
    in_maps = make_in_maps(hidden_states, wq, wk, wv, wo)
    res = run_bass_kernel_spmd(nc, in_maps, core_ids=list(range(NCORES)))
    return combine_outputs([r["out"] for r in res.results])

